# revision 56
# baseline (speedup 1.0000x reference)
"""Trainium2 Bass kernel for nn_AttentionModule (channel-attention block).

Reference computation (per example):
    q = wq @ x + bq        # [C, P]  (1x1 conv == channelwise linear)
    k = wk @ x + bk
    v = x                  # [C, P]
    att[n] = softmax((q[n] @ k[n].T) / sqrt(dh))   # [dh, dh] per head
    out = wo @ (att @ v) + bo + x

Sharding: pure data parallel -- B=16 examples, 2 per core across 8 cores;
weights replicated. No collectives.

Gram-matrix reformulation (the key FLOP cut vs the direct q/k path):
    logits = q @ k.T = wq S wk^T + bq (wk s + P bk)^T + (wq s) bk^T
  with S = x @ x^T [C, C] and s = x @ 1_P. Computing S once (C^2 P MACs)
  replaces both q and k projections (2 C^2 P) and the P-wide attention
  contraction. S is symmetric, so only upper-triangle blocks are
  computed (1280 of 2048 N-columns per p-tile); the missing blocks are
  mirrored with cheap PE transposes. Per-example MACs ~1.9e9 vs the
  direct path's ~3.54e9. The rank-2 bias correction rides into the
  logits PSUM group as one K=2 matmul of host-precomputed rows
  (bq, qs) x (ks + P bk, bk).

Kernel design (per core; bf16 matmuls, f32 PSUM):
  * S accumulates in 4 PSUM banks over 32 xT p-tiles. xT ships
    quad-packed [8, 128, 2048] (four p-tiles per SBUF tile) so one
    large DMA covers four tiles; ALL tiles are issued upfront so the
    HW DGE queues never run dry during the DMA-paced S0 phase. Both
    examples' S phases end with the last four p-tiles in ci-major
    order (mm_s_tail) with the extraction strips emitted right
    behind, so bank ci extracts while bank ci+1 still accumulates.
  * S -> SBUF as single bf16 (hi/lo refinement measured +2e-3 rel err
    for ~10K PE cycles/example; tolerance is 2e-2 and this sits at
    1.1e-2) in 128-col strips split across ACT and DVE; V = S @ wkT
    uses S's symmetry (row blocks serve as lhsT directly), mirrors
    interleave with V matmul groups.
  * logits per head-pair tile [128, 128]: 16 matmuls + ONE K=8 bias
    matmul (k-row 2t+j = bias row j of pair t, rhs block-diagonal);
    all four pair banks share one PSUM bank as one accumulation group.
  * softmax: split emit_att / emit_g. Engines execute their queues IN
    ORDER, so the non-PE chain (ACT EXPs without accum_out -- DVE
    free-dim reduces give Z instead, saving 281ns READ_ACCUMULATOR per
    half on ACT's serial path; GpSimd zero-fills; DVE recip+scale) is
    emitted EARLY while the PE-consuming G matmuls are emitted late,
    after the conv chunks the chain hides under. G = att_de @ woT + I.
  * epilogue: out = G^T @ x + bo as 8 chunks x 4 co of N=512 matmuls
    per example, rhs from full-row [128, 4096] x tiles. PSUM->SBUF
    bias copies alternate DVE/ACT; the last e1 chunk's parity is
    flipped so the final co extracts on DVE (shorter chain to the
    last drain). e1 outputs drain in stages across both queues for a
    short exit tail; e0's drain as two large Sync-only stages.
  * DMA issue discipline (the big scheduling lever): only Sync and
    Scalar drive the hardware DGE. A dma_start costs ~600ns of
    issuing-engine time AND can block for MANY microseconds waiting
    for a free DMA-completion semaphore slot while its queue streams
    bulk transfers. So Scalar (=ACT) carries input DMA issues only
    BEFORE any queued ACT compute; everything issued after the S0
    strips rides Sync, where blocking is free. Queue ORDER also
    matters: xT1's first tiles precede the 4MB of x0 on Sync's queue
    (S1 p0 needs them ~10us before x0's first use).
  * schedule: S1 p-tiles interleave with conv0 chunks (6/6/4 + tail);
    e1's strip/V/logits/softmax chains ride behind conv0 chunks 3-5
    (whose extraction is forced onto DVE to keep ACT clear), G1 after
    conv0, then conv1.

Measured on trn2 (8 cores): ~123.5 us exec in a cool window, ~126-129
under accumulated power throttling (prior Gram kernel: ~132 us,
direct-path baseline: ~227 us), rel err 1.1e-2 vs f32 reference. PE busy ~107.5us of 600 matmuls at ~2.24 col/ns (93% of
bf16 peak); mid-kernel PE gaps ~1.5us; the rest is the fixed ~7.2us
Tile prologue (engine iram loads + semaphore barrier), ~2.4us DMA
cold-start to the first matmul, and ~5.5us exit (last drains + the
hardwired drain/barrier epilogue). fp8 DoubleRow for the epilogue was
prototyped and measured numerically unaffordable: e4m3 on both
operands adds ~2.6e-2 rel err (attention here is near-one-hot,
alpha=0.69), blowing the 2e-2 tolerance.
"""

import numpy as np
import ml_dtypes

BF = np.dtype(ml_dtypes.bfloat16)

import concourse.bass as bass
import concourse.tile as tile
from concourse import bacc, mybir
from concourse import bass_utils

F32 = mybir.dt.float32
BF16 = mybir.dt.bfloat16
EXP = mybir.ActivationFunctionType.Exp
IDENT = mybir.ActivationFunctionType.Identity

B, C, HH, WW = 16, 512, 64, 64
P = HH * WW            # 4096 spatial positions
NCORES = 8
BL = B // NCORES       # 2 examples per core
NH = 8
DH = C // NH           # 64
NPT = P // 128         # 32 p-tiles (S accumulation granularity)
NPG = NPT // 2         # xT tile count x2 (tiles are quad-packed: NPG//2 = 8)
NP5 = P // 512         # 8 512-wide chunks (epilogue granularity)
NCT = C // 128         # 4 channel tiles
WCOLS = NCT * C        # 2048 cols per packed weight


def build_nc():
    nc = bacc.Bacc(
        "TRN2", target_bir_lowering=False, debug=False, enable_asserts=False
    )
    xt_d = nc.dram_tensor("xt", [BL, NPG // 2, 128, 2048], BF16,
                          kind="ExternalInput").ap()
    x_d = nc.dram_tensor("x", [BL, C, P], BF16, kind="ExternalInput").ap()
    wpack_d = nc.dram_tensor("wpack", [128, 3 * WCOLS + 192], BF16,
                             kind="ExternalInput").ap()
    bias2_d = nc.dram_tensor("bias2", [8, BL * 640], BF16,
                             kind="ExternalInput").ap()
    bpack_d = nc.dram_tensor("bpack", [128, NCT], F32,
                             kind="ExternalInput").ap()
    out_d = nc.dram_tensor("out", [BL, C, P], BF16, kind="ExternalOutput").ap()

    with (
        tile.TileContext(nc) as tc,
        tc.tile_pool(name="w", bufs=1) as wpool,
        tc.tile_pool(name="xt", bufs=10) as xtpool,
        tc.tile_pool(name="x", bufs=8) as xpool,
        tc.tile_pool(name="slv", bufs=8) as slvpool,
        tc.tile_pool(name="pair", bufs=8) as pairpool,
        tc.tile_pool(name="z", bufs=16) as zpool,
        tc.tile_pool(name="g", bufs=8) as gpool,
        tc.tile_pool(name="o2r", bufs=8) as o2rpool,
        tc.tile_pool(name="sp", bufs=4, space="PSUM") as spool,
        tc.tile_pool(name="attp", bufs=1, space="PSUM") as attpool,
        tc.tile_pool(name="p2p", bufs=3, space="PSUM") as p2pool,
    ):
        # ---- resident weights / biases -------------------------------
        wk_t = wpool.tile([128, WCOLS], BF16, tag="wk")
        wq_t = wpool.tile([128, WCOLS], BF16, tag="wq")
        wo_t = wpool.tile([128, WCOLS], BF16, tag="wo")
        konst = wpool.tile([128, 192], BF16, tag="konst")
        bias2 = wpool.tile([8, BL * 640], BF16, tag="bias2")
        bo_t = wpool.tile([128, NCT], F32, tag="bo")
        zblk = konst[:, 0:64]     # all-zeros [128, 64]
        eye = konst[:, 64:192]    # identity [128, 128]
        shift = wpool.tile([128, 1], F32, tag="shift")
        nc.gpsimd.memset(shift[:], -55.0)

        # weight DMAs (issued off Scalar). Deferred so they don't steal
        # HBM bandwidth from the xt0 stream that feeds S0 (the S phase
        # needs ~250GB/s for xt alone): konst (48KB, feeds the mirror
        # eye) early, wk/wq late in S0, and bias2/wo/bo after the S0
        # loop entirely (first consumers: logits0 ~40us, G0 ~44us,
        # conv0 extraction ~47us).
        wdmas = {3: (konst[:], wpack_d[:, 3 * WCOLS: 3 * WCOLS + 192]),
                 16: (wk_t[:], wpack_d[:, WCOLS: 2 * WCOLS]),
                 18: (bias2[:], bias2_d[:]),
                 20: (bo_t[:], bpack_d[:]),
                 22: (wq_t[:], wpack_d[:, 0: WCOLS])}
        wdmas_late = [(wo_t[:], wpack_d[:, 2 * WCOLS: 3 * WCOLS])]

        def dma_xtg(e, j, tiles, stripes=1, engines=None):
            """Issue the DMA(s) for one quad-packed xT tile [128, 2048]
            covering p-tiles 4j..4j+3 (contiguous in DRAM, so a single
            large DMA per tile keeps the HW DGE queues streaming)."""
            if engines is None:
                engines = (nc.sync,)
            xtt = xtpool.tile([128, 2048], BF16, tag="xt", name=f"xt{e}_{j}")
            w = 2048 // stripes
            for st in range(stripes):
                engines[st % len(engines)].dma_start(
                    xtt[:, st * w:(st + 1) * w],
                    xt_d[e, j, :, st * w:(st + 1) * w])
            tiles[j] = xtt

        def mm_s(Sb, tiles, p):
            """Upper-triangle Gram matmuls for one p-tile. tiles may
            carry a dedicated first-p-tile tile under key -1 (a small
            single-queue DMA that unblocks the very first matmul at the
            earliest possible moment)."""
            if p == 0 and -1 in tiles:
                xtt, base = tiles[-1], 0
            else:
                xtt, base = tiles[p // 4], (p % 4) * 512
            for ci in range(NCT):
                nc.tensor.matmul(Sb[ci][:],
                                 xtt[:, base + 128 * ci: base + 128 * (ci + 1)],
                                 xtt[:, base + 128 * ci: base + 512],
                                 start=(p == 0), stop=(p == NPT - 1))

        def mm_s_tail(Sb, tiles):
            """Last four p-tiles in ci-major order: bank ci stops several
            matmuls before bank ci+1, so hi-strip extraction (ACT)
            overlaps the S tail instead of serializing after it."""
            for ci in range(NCT):
                for p in range(NPT - 4, NPT):
                    base = (p % 4) * 512
                    xtt = tiles[p // 4]
                    nc.tensor.matmul(Sb[ci][:],
                                     xtt[:, base + 128 * ci: base + 128 * (ci + 1)],
                                     xtt[:, base + 128 * ci: base + 512],
                                     start=False, stop=(p == NPT - 1))

        def emit_x_chunk(e, xch, ci, engine):
            """One full-row [128, 4096] load of the [C, P]-layout x
            (epilogue rhs): 1MB with 8KB contiguous lines, one DMA per
            ci so issue-engine time stays low."""
            xt = xpool.tile([128, P], BF16, tag="x", name=f"x{e}_{ci}")
            engine.dma_start(xt[:], x_d[e, ci * 128:(ci + 1) * 128, :])
            xch[ci] = xt

        def emit_s_extract(e, Sb):
            """S extraction (single bf16; rel err ~1.1e-2 vs tolerance
            2e-2 -- the hi/lo split costs 10K PE cycles/example for
            ~2e-3). Strips split across ACT and DVE so extraction
            wall-time halves (both engines can read PSUM; GpSimd can't).
            Split out from the mirror/V stage so the strip chain can be
            issued early and hide under unrelated PE work."""
            Shi = [slvpool.tile([128, C], BF16, tag="slv", name=f"Shi{e}_{ci}")
                   for ci in range(NCT)]
            for ci in range(NCT):
                for s in range(NCT - ci):
                    dsl = slice((ci + s) * 128, (ci + s + 1) * 128)
                    ssl = slice(s * 128, (s + 1) * 128)
                    if (ci + s) % 2 == 0:
                        nc.scalar.activation(Shi[ci][:, dsl], Sb[ci][:, ssl],
                                             IDENT)
                    else:
                        nc.vector.tensor_copy(Shi[ci][:, dsl], Sb[ci][:, ssl])
            return Shi

        def emit_v(e, Shi):
            """Symmetry mirrors + V = S @ wkT + V extraction."""
            def mirror(i, j):
                # S[i-block, j-block] = S[j-block, i-block]^T for j < i
                tp = p2pool.tile([128, 128], BF16, tag="p2",
                                 name=f"mt{e}_{i}{j}")
                nc.tensor.transpose(tp[:], Shi[j][:, 128 * i:128 * (i + 1)],
                                    eye[:])
                nc.vector.tensor_copy(Shi[i][:, 128 * j:128 * (j + 1)], tp[:])

            Vb = [spool.tile([128, 512], F32, tag="sp", name=f"V{e}_{ci}")
                  for ci in range(NCT)]

            def vmm(cj):
                for ci in range(NCT):
                    nc.tensor.matmul(Vb[ci][:],
                                     Shi[cj][:, 128 * ci:128 * (ci + 1)],
                                     wk_t[:, C * cj:C * (cj + 1)],
                                     start=(cj == 0), stop=(cj == NCT - 1))

            mirror(1, 0)
            vmm(0)
            mirror(2, 0)
            mirror(2, 1)
            vmm(1)
            mirror(3, 0)
            mirror(3, 1)
            mirror(3, 2)
            vmm(2)
            vmm(3)

            Vhi = [slvpool.tile([128, C], BF16, tag="slv", name=f"Vhi{e}_{ci}")
                   for ci in range(NCT)]
            for ci in range(NCT):
                for s in range(2):
                    sl = slice(s * 256, (s + 1) * 256)
                    if (ci + s) % 2 == 0:
                        nc.scalar.activation(Vhi[ci][:, sl], Vb[ci][:, sl],
                                             IDENT)
                    else:
                        nc.vector.tensor_copy(Vhi[ci][:, sl], Vb[ci][:, sl])
            return Vhi

        def emit_logits(e, Vhi):
            """Per head-pair logit banks [d, e']: wq^T V + rank-2 bias.
            All four pair banks share one PSUM bank (2KB) as one group.
            The four per-pair rank-2 bias corrections are packed into ONE
            K=8 matmul: k-row 2t+j carries bias row j of pair t, with the
            rhs zeroed outside pair t's 128-col block (block-diagonal)."""
            bt = attpool.tile([128, 512], F32, tag="attp", name=f"Lb{e}")
            banks = [bt[:, t * 128:(t + 1) * 128] for t in range(4)]
            for cj in range(NCT):
                for t in range(4):
                    nc.tensor.matmul(banks[t][:],
                                     wq_t[:, C * cj + 128 * t: C * cj + 128 * (t + 1)],
                                     Vhi[cj][:, 128 * t:128 * (t + 1)],
                                     start=(cj == 0 and t == 0), stop=False)
            nc.tensor.matmul(bt[:],
                             bias2[:, e * 640: e * 640 + 128],
                             bias2[:, e * 640 + 128: (e + 1) * 640],
                             start=False, stop=True)
            return banks

        def emit_att(e, banks):
            """Softmax chain -> normalized att_de tiles. Engines execute
            their queues IN ORDER, so this non-PE part is emitted early
            (its ACT/DVE/Pool ops queue ahead of later conv extractions)
            while the PE-consuming emit_g is emitted late, after the
            conv chunks the chain should hide under. EXPs carry no
            accum_out (the 281ns ACTIVATION_READ_ACCUMULATOR per half
            would double ACT's serial time); Z is a cheap DVE free-dim
            reduce; zero-quadrant fills ride the idle GpSimd."""
            att_des = []
            for t in range(4):
                bank = banks[t]
                pr = pairpool.tile([128, 128], BF16, tag="pair", name=f"pr{e}_{t}")
                z = zpool.tile([128, 1], F32, tag="z", name=f"z{e}_{t}")
                nc.gpsimd.tensor_copy(pr[0:64, 64:128], zblk[0:64, :])
                nc.gpsimd.tensor_copy(pr[64:128, 0:64], zblk[64:128, :])
                nc.scalar.activation(pr[0:64, 0:64], bank[0:64, 0:64], EXP,
                                     scale=0.125, bias=shift[0:64, :])
                nc.scalar.activation(pr[64:128, 64:128], bank[64:128, 64:128],
                                     EXP, scale=0.125, bias=shift[64:128, :])
                nc.vector.tensor_reduce(z[0:64, :], pr[0:64, 0:64],
                                        mybir.AxisListType.X,
                                        mybir.AluOpType.add)
                nc.vector.tensor_reduce(z[64:128, :], pr[64:128, 64:128],
                                        mybir.AxisListType.X,
                                        mybir.AluOpType.add)
                rz = zpool.tile([128, 1], F32, tag="z", name=f"rz{e}_{t}")
                nc.vector.reciprocal(rz[:], z[:])
                att_de = pairpool.tile([128, 128], BF16, tag="pair",
                                       name=f"attde{e}_{t}")
                nc.vector.tensor_scalar_mul(att_de[:], pr[:], rz[:, 0:1])
                att_des.append(att_de)
            return att_des

        def emit_g(e, att_des):
            """G = wo att + I matmuls + extraction, interleaved per pair
            so casts pipeline behind the next pair's matmuls."""
            gs = []
            for t in range(4):
                gp = p2pool.tile([128, 512], F32, tag="p2", name=f"gp{e}_{t}")
                nc.tensor.matmul(gp[:], att_des[t][:],
                                 wo_t[:, C * t:C * (t + 1)],
                                 start=True, stop=False)
                nc.tensor.matmul(gp[:, 128 * t:128 * (t + 1)], eye[:], eye[:],
                                 start=False, stop=True)
                g = gpool.tile([128, C], BF16, tag="g", name=f"g{e}_{t}")
                if t % 2 == 0:
                    nc.vector.tensor_copy(g[:], gp[:])
                else:
                    nc.scalar.activation(g[:], gp[:], IDENT)
                gs.append(g)
            return gs

        def emit_o2rows(e):
            return [o2rpool.tile([128, P], BF16, tag="o2r", name=f"o2r{e}_{co}")
                    for co in range(NCT)]

        def emit_conv_chunk(e, xch, gs, o2rows, p5, ext="alt"):
            """ext='vector' forces PSUM->SBUF extraction onto DVE for
            chunks that ride alongside e1's sv/logits chain, keeping the
            ACT engine free for that chain's strip extractions. e0's
            output drains all issue from Sync so the Scalar engine's
            queue stays clear for e1's softmax chain; e1's split across
            both queues for a short exit tail."""
            drain = (nc.sync, nc.sync) if e == 0 else (nc.sync, nc.scalar)
            sl = slice(p5 * 512, (p5 + 1) * 512)
            for co in range(NCT):
                o2p = p2pool.tile([128, 512], F32, tag="p2",
                                  name=f"o2p{e}_{p5}_{co}")
                for et in range(NCT):
                    nc.tensor.matmul(
                        o2p[:],
                        gs[et][:, co * 128:(co + 1) * 128],
                        xch[et][:, sl],
                        start=(et == 0), stop=(et == NCT - 1))
                # PSUM->SBUF + bias split between DVE and ACT; the very
                # last chunk's parity is flipped so the final co lands
                # on DVE (its chain to the last drain is ~1us shorter
                # than via ACT, which still holds the prior extraction)
                par = (p5 * NCT + co + (1 if e == 1 and p5 == 7 else 0)) % 2
                if ext == "vector" or par == 0:
                    nc.vector.tensor_scalar_add(o2rows[co][:, sl], o2p[:],
                                                bo_t[:, co:co + 1])
                else:
                    nc.scalar.activation(o2rows[co][:, sl], o2p[:], IDENT,
                                         bias=bo_t[:, co:co + 1])
                # e1: last two stages drain per-co right behind the copy
                # so the exit tail is one 64KB transfer per queue deep
                if e == 1 and p5 >= 6:
                    drain[co % 2].dma_start(
                        out_d[e, co * 128:(co + 1) * 128, sl], o2rows[co][:, sl])
            # staged output drains: large per-co DMAs; e0's all ride Sync
            # in two halves (its tail is hidden), e1's stay fine-grained
            # across both queues for a short exit tail
            if e == 0:
                if p5 == 3 or p5 == 7:
                    h = slice(0, 2048) if p5 == 3 else slice(2048, 4096)
                    for co in range(NCT):
                        nc.sync.dma_start(
                            out_d[e, co * 128:(co + 1) * 128, h],
                            o2rows[co][:, h])
            elif p5 == 3:
                for co in range(NCT):
                    drain[co % 2].dma_start(
                        out_d[e, co * 128:(co + 1) * 128, 0:2048],
                        o2rows[co][:, 0:2048])
            elif p5 == 5:
                for co in range(NCT):
                    drain[(co + 1) % 2].dma_start(
                        out_d[e, co * 128:(co + 1) * 128, 2048:3072],
                        o2rows[co][:, 2048:3072])


        # ---- schedule -------------------------------------------------
        # e0 S phase: xT0 rides BOTH hardware DMA queues (sync+scalar),
        # ALL tiles issued upfront so the DGE queues never run dry
        # during the DMA-paced S0 phase; weights ride scalar behind the
        # xt0 stream, x0 late
        Sb0 = [spool.tile([128, 512 - 128 * ci], F32, tag="sp",
                          name=f"S0_{ci}") for ci in range(NCT)]
        xt0 = {}
        xch0 = [None] * NCT
        both = (nc.sync, nc.scalar)
        dma_xtg(0, 0, xt0, stripes=4, engines=both)
        dma_xtg(0, 1, xt0, stripes=2, engines=both)
        # j2..j7 striped on a single queue each: halves complete
        # sequentially, so each tile's first two p-tiles unblock a
        # 256KB transfer earlier (the S0 phase is DMA-paced; 512KB
        # all-or-nothing arrivals stall the PE in bursts)
        for j in range(2, 8):
            dma_xtg(0, j, xt0, stripes=2, engines=(both[j % 2],))
        # Queue-order matters as much as issue time: xT1 j0-j2 go FIRST
        # on Sync's queue (S1 p0 needs j0 at ~34us; behind 4MB of x0 it
        # would land ~37), then x0 split across both queues, then the
        # late weights. Scalar-ENGINE issues sit before any ACT compute
        # (strips come later), so a sem-slot block there is harmless.
        xt1 = {}
        for p in range(NPT - 4):
            mm_s(Sb0, xt0, p)
            if p in wdmas:
                dst, src = wdmas.pop(p)
                nc.scalar.dma_start(dst, src)
            if 21 <= p < 24:
                dma_xtg(1, p - 21, xt1)
            elif p == 24:
                # wo ahead of x0 on Sync's queue: the G0 matmuls (which
                # gate gs0 and thus conv0's start) need wo by ~43.5us,
                # while x0's first use (conv0 chunk 0) is ~46.5us
                nc.sync.dma_start(*wdmas_late[0])
            elif 25 <= p <= 27:
                # x0 c1-c3 ride Scalar's queue, which drains by ~38us
                # while Sync is still packed (issues are pre-strips, so
                # a sem-slot block cannot delay ACT compute)
                emit_x_chunk(0, xch0, p - 24, nc.scalar)
        # S0 tail in ci-major order with strips right behind: bank ci's
        # extraction starts as soon as it stops, ahead of everything
        # else in ACT's in-order queue
        mm_s_tail(Sb0, xt0)
        Shi0 = emit_s_extract(0, Sb0)
        # last x0 row on Sync (no-blocking rule: never put a blockable
        # DMA issue on Scalar once ACT compute is queued)
        emit_x_chunk(0, xch0, 0, nc.sync)

        Vhi0 = emit_v(0, Shi0)
        Sb1 = [spool.tile([128, 512 - 128 * ci], F32, tag="sp",
                          name=f"S1_{ci}") for ci in range(NCT)]
        for p in range(0, 6):
            mm_s(Sb1, xt1, p)
        banks0 = emit_logits(0, Vhi0)
        att0 = emit_att(0, banks0)
        for p in range(6, 12):
            mm_s(Sb1, xt1, p)
        gs0 = emit_g(0, att0)

        # conv0 chunks interleaved with remaining e1 S tiles + x1 loads;
        # e1's extraction/logits/softmax chains each ride behind a conv0
        # chunk so their serial ACT/DVE latency hides under PE work
        o2r0 = emit_o2rows(0)
        xch1 = [None] * NCT
        for j in range(3, 6):
            dma_xtg(1, j, xt1)
        p1 = 12
        x1_next = 0
        sched = [6, 6, 4, 0, 0, 0, 0, 0]
        for i in range(NP5):
            if i == 3:
                # S1 tail in ci-major order (bank ci stops early) so the
                # strip chain starts before the conv chunk and hides
                # under conv3/conv4's PE work; strips are queued on
                # ACT/DVE ahead of conv3's extraction ops
                mm_s_tail(Sb1, xt1)
                Shi1 = emit_s_extract(1, Sb1)
            # chunks riding alongside e1's sv chain extract on DVE only,
            # so ACT's queue stays clear for the chain's strips
            emit_conv_chunk(0, xch0, gs0, o2r0, i,
                            ext="vector" if 3 <= i <= 4 else "alt")
            if i < 2:
                dma_xtg(1, 6 + i, xt1)
            for _ in range(sched[i]):
                if p1 < NPT - 4:
                    mm_s(Sb1, xt1, p1)
                    p1 += 1
            if x1_next < 4:
                emit_x_chunk(1, xch1, x1_next, nc.sync)
                x1_next += 1
            if i == 4:
                Vhi1 = emit_v(1, Shi1)
                banks1 = emit_logits(1, Vhi1)
            elif i == 5:
                # non-PE softmax chain only: its ACT/DVE ops queue ahead
                # of conv5-7's extractions; the PE-consuming G matmuls
                # are emitted after conv7 so PE never blocks on att_de
                att1 = emit_att(1, banks1)

        gs1 = emit_g(1, att1)
        o2r1 = emit_o2rows(1)
        for i in range(NP5):
            emit_conv_chunk(1, xch1, gs1, o2r1, i)

    nc.compile()
    return nc


_NC_CACHE = None


def _get_nc():
    global _NC_CACHE
    if _NC_CACHE is None:
        _NC_CACHE = build_nc()
    return _NC_CACHE


def make_in_maps(inputs):
    x = np.ascontiguousarray(np.asarray(inputs["x"], dtype=np.float32))
    wq = np.asarray(inputs["wq"], dtype=np.float32)
    wk = np.asarray(inputs["wk"], dtype=np.float32)
    wo = np.asarray(inputs["wo"], dtype=np.float32)
    bq = np.asarray(inputs["bq"], dtype=np.float32)
    bk = np.asarray(inputs["bk"], dtype=np.float32)
    bo = np.asarray(inputs["bo"], dtype=np.float32)

    x32 = x.reshape(B, C, P)
    xr = x32.astype(BF)                                   # [B, C, P] bf16
    xtr = np.ascontiguousarray(xr.transpose(0, 2, 1))     # [B, P, C] bf16
    # quad-packed xT: [B, NPG//2, 128, 2048], tile j = p-tiles 4j..4j+3
    xt4 = np.ascontiguousarray(
        xtr.reshape(B, NPG // 2, 4, 128, C).transpose(0, 1, 3, 2, 4)
           .reshape(B, NPG // 2, 128, 2048))

    # rank-2 bias-correction rows (exact f32 host math)
    s = x32.sum(axis=2)                                   # [B, C]
    qs = s @ wq.T                                         # [B, C]
    ks = s @ wk.T                                         # [B, C]

    wpack = np.zeros((128, 3 * WCOLS + 192), dtype=BF)
    for i, w in enumerate((wq, wk, wo)):
        wt = w.T.astype(BF)  # [ci, co]
        for ci in range(NCT):
            wpack[:, i * WCOLS + ci * C: i * WCOLS + (ci + 1) * C] = \
                wt[ci * 128:(ci + 1) * 128, :]
    ko = 3 * WCOLS
    wpack[:, ko + 64: ko + 192] = np.eye(128, dtype=np.float32).astype(BF)

    bpack = bo.reshape(NCT, 128).T.astype(np.float32)
    bpack = np.ascontiguousarray(bpack)

    in_maps = []
    for cix in range(NCORES):
        # K=8 packed rank-2 bias rows: k-row 2t+j holds bias row j of
        # head-pair t; lhsT cols are the pair's 128 d-rows, rhs cols are
        # zero outside the pair's 128-col block (block-diagonal).
        bias2 = np.zeros((8, BL * 640), dtype=BF)
        for e in range(BL):
            ge = cix * BL + e
            a = np.stack([bq, qs[ge]])                 # [2, C] lhsT rows
            b = np.stack([ks[ge] + P * bk, bk])        # [2, C] rhs rows
            for t in range(4):
                for j in range(2):
                    k = 2 * t + j
                    bias2[k, e * 640: e * 640 + 128] = \
                        a[j, t * 128:(t + 1) * 128].astype(BF)
                    bias2[k, e * 640 + 128 + t * 128: e * 640 + 128 + (t + 1) * 128] = \
                        b[j, t * 128:(t + 1) * 128].astype(BF)
        in_maps.append({
            "x": np.ascontiguousarray(xr[cix * BL: (cix + 1) * BL]),
            "xt": np.ascontiguousarray(xt4[cix * BL: (cix + 1) * BL]),
            "wpack": wpack, "bias2": bias2, "bpack": bpack,
        })
    return in_maps


def run_sharded(inputs, trace=False, **kw):
    nc = _get_nc()
    in_maps = make_in_maps(inputs)
    res = bass_utils.run_bass_kernel_spmd(
        nc, in_maps, core_ids=list(range(NCORES)), trace=trace, **kw
    )
    outs = [np.asarray(res.results[i]["out"]).astype(np.float32)
            for i in range(NCORES)]
    full = np.concatenate(outs, axis=0).reshape(B, C, HH, WW)
    return full.astype(np.float32), res


def kernel(**inputs):
    out, _ = run_sharded(inputs, trace=False)
    return out



# revision 57
# speedup vs baseline: 1.0508x; 1.0508x over previous
"""Trainium2 Bass kernel for nn_AttentionModule (channel-attention block).

Reference computation (per example):
    q = wq @ x + bq        # [C, P]  (1x1 conv == channelwise linear)
    k = wk @ x + bk
    v = x                  # [C, P]
    att[n] = softmax((q[n] @ k[n].T) / sqrt(dh))   # [dh, dh] per head
    out = wo @ (att @ v) + bo + x

Sharding: pure data parallel -- B=16 examples, 2 per core across 8 cores;
weights replicated. No collectives.

Gram-matrix reformulation (the key FLOP cut vs the direct q/k path):
    logits = q @ k.T = wq S wk^T + bq (wk s + P bk)^T + (wq s) bk^T
  with S = x @ x^T [C, C] and s = x @ 1_P. Computing S once (C^2 P MACs)
  replaces both q and k projections (2 C^2 P) and the P-wide attention
  contraction. S is symmetric, so only upper-triangle blocks are
  computed (1280 of 2048 N-columns per p-tile); the missing blocks are
  mirrored with cheap PE transposes. Per-example MACs ~1.9e9 vs the
  direct path's ~3.54e9. The rank-2 bias correction rides into the
  logits PSUM group as one K=2 matmul of host-precomputed rows
  (bq, qs) x (ks + P bk, bk).

Kernel design (per core; bf16 matmuls, f32 PSUM):
  * S accumulates in 4 PSUM banks over 32 xT p-tiles. xT ships
    quad-packed [8, 128, 2048] (four p-tiles per SBUF tile) so one
    large DMA covers four tiles; ALL tiles are issued upfront so the
    HW DGE queues never run dry during the DMA-paced S0 phase. Both
    examples' S phases end with the last four p-tiles in ci-major
    order (mm_s_tail) with the extraction strips emitted right
    behind, so bank ci extracts while bank ci+1 still accumulates.
  * S -> SBUF as single bf16 (hi/lo refinement measured +2e-3 rel err
    for ~10K PE cycles/example; tolerance is 2e-2 and this sits at
    1.1e-2) in 128-col strips split across ACT and DVE; V = S @ wkT
    uses S's symmetry (row blocks serve as lhsT directly), mirrors
    interleave with V matmul groups.
  * logits per head-pair tile [128, 128]: 16 matmuls + ONE K=8 bias
    matmul (k-row 2t+j = bias row j of pair t, rhs block-diagonal);
    all four pair banks share one PSUM bank as one accumulation group.
  * softmax: split emit_att / emit_g. Engines execute their queues IN
    ORDER, so the non-PE chain (ACT EXPs without accum_out -- DVE
    free-dim reduces give Z instead, saving 281ns READ_ACCUMULATOR per
    half on ACT's serial path; GpSimd zero-fills; DVE recip+scale) is
    emitted EARLY while the PE-consuming G matmuls are emitted late,
    after the conv chunks the chain hides under. G = att_de @ woT + I.
  * epilogue: out = G^T @ x + bo as 8 chunks x 4 co of N=512 matmuls
    per example, rhs from full-row [128, 4096] x tiles. PSUM->SBUF
    bias copies alternate DVE/ACT; the last e1 chunk's parity is
    flipped so the final co extracts on DVE (shorter chain to the
    last drain). e1 outputs drain in stages across both queues for a
    short exit tail; e0's drain as two large Sync-only stages.
  * DMA issue discipline (the big scheduling lever): only Sync and
    Scalar drive the hardware DGE. A dma_start costs ~600ns of
    issuing-engine time AND can block for MANY microseconds waiting
    for a free DMA-completion semaphore slot while its queue streams
    bulk transfers. So Scalar (=ACT) carries input DMA issues only
    BEFORE any queued ACT compute; everything issued after the S0
    strips rides Sync, where blocking is free. Queue ORDER also
    matters: xT1's first tiles precede the 4MB of x0 on Sync's queue
    (S1 p0 needs them ~10us before x0's first use).
  * schedule: S1 p-tiles interleave with conv0 chunks (6/6/4 + tail);
    e1's strip/V/logits/softmax chains ride behind conv0 chunks 3-5
    (whose extraction is forced onto DVE to keep ACT clear), G1 after
    conv0, then conv1.

Measured on trn2 (8 cores): ~123.5 us exec in a cool window, ~126-129
under accumulated power throttling (prior Gram kernel: ~132 us,
direct-path baseline: ~227 us), rel err 1.1e-2 vs f32 reference. PE busy ~107.5us of 600 matmuls at ~2.24 col/ns (93% of
bf16 peak); mid-kernel PE gaps ~1.5us; the rest is the fixed ~7.2us
Tile prologue (engine iram loads + semaphore barrier), ~2.4us DMA
cold-start to the first matmul, and ~5.5us exit (last drains + the
hardwired drain/barrier epilogue). fp8 DoubleRow for the epilogue was
prototyped and measured numerically unaffordable: e4m3 on both
operands adds ~2.6e-2 rel err (attention here is near-one-hot,
alpha=0.69), blowing the 2e-2 tolerance.
"""

import numpy as np
import ml_dtypes

BF = np.dtype(ml_dtypes.bfloat16)

import concourse.bass as bass
import concourse.tile as tile
from concourse import bacc, mybir
from concourse import bass_utils

F32 = mybir.dt.float32
BF16 = mybir.dt.bfloat16
EXP = mybir.ActivationFunctionType.Exp
IDENT = mybir.ActivationFunctionType.Identity

B, C, HH, WW = 16, 512, 64, 64
P = HH * WW            # 4096 spatial positions
NCORES = 8
BL = B // NCORES       # 2 examples per core
NH = 8
DH = C // NH           # 64
NPT = P // 128         # 32 p-tiles (S accumulation granularity)
NPG = NPT // 2         # xT tile count x2 (tiles are quad-packed: NPG//2 = 8)
NP5 = P // 512         # 8 512-wide chunks (epilogue granularity)
NCT = C // 128         # 4 channel tiles
WCOLS = NCT * C        # 2048 cols per packed weight


def build_nc():
    nc = bacc.Bacc(
        "TRN2", target_bir_lowering=False, debug=False, enable_asserts=False
    )
    xt_d = nc.dram_tensor("xt", [BL, NPG // 2, 128, 2048], BF16,
                          kind="ExternalInput").ap()
    x_d = nc.dram_tensor("x", [BL, C, P], BF16, kind="ExternalInput").ap()
    wpack_d = nc.dram_tensor("wpack", [128, 3 * WCOLS + 192], BF16,
                             kind="ExternalInput").ap()
    bias2_d = nc.dram_tensor("bias2", [8, BL * 640], BF16,
                             kind="ExternalInput").ap()
    bpack_d = nc.dram_tensor("bpack", [128, NCT], F32,
                             kind="ExternalInput").ap()
    out_d = nc.dram_tensor("out", [BL, C, P], BF16, kind="ExternalOutput").ap()

    with (
        tile.TileContext(nc) as tc,
        tc.tile_pool(name="w", bufs=1) as wpool,
        tc.tile_pool(name="xt", bufs=10) as xtpool,
        tc.tile_pool(name="x", bufs=8) as xpool,
        tc.tile_pool(name="slv", bufs=8) as slvpool,
        tc.tile_pool(name="pair", bufs=8) as pairpool,
        tc.tile_pool(name="z", bufs=16) as zpool,
        tc.tile_pool(name="g", bufs=8) as gpool,
        tc.tile_pool(name="o2r", bufs=8) as o2rpool,
        tc.tile_pool(name="sp", bufs=4, space="PSUM") as spool,
        tc.tile_pool(name="attp", bufs=1, space="PSUM") as attpool,
        tc.tile_pool(name="p2p", bufs=3, space="PSUM") as p2pool,
    ):
        # ---- resident weights / biases -------------------------------
        wk_t = wpool.tile([128, WCOLS], BF16, tag="wk")
        wq_t = wpool.tile([128, WCOLS], BF16, tag="wq")
        wo_t = wpool.tile([128, WCOLS], BF16, tag="wo")
        konst = wpool.tile([128, 192], BF16, tag="konst")
        bias2 = wpool.tile([8, BL * 640], BF16, tag="bias2")
        bo_t = wpool.tile([128, NCT], F32, tag="bo")
        zblk = konst[:, 0:64]     # all-zeros [128, 64]
        eye = konst[:, 64:192]    # identity [128, 128]
        shift = wpool.tile([128, 1], F32, tag="shift")
        nc.gpsimd.memset(shift[:], -55.0)

        # weight DMAs (issued off Scalar). Deferred so they don't steal
        # HBM bandwidth from the xt0 stream that feeds S0 (the S phase
        # needs ~250GB/s for xt alone): konst (48KB, feeds the mirror
        # eye) early, wk/wq late in S0, and bias2/wo/bo after the S0
        # loop entirely (first consumers: logits0 ~40us, G0 ~44us,
        # conv0 extraction ~47us).
        wdmas = {3: (konst[:], wpack_d[:, 3 * WCOLS: 3 * WCOLS + 192]),
                 16: (wk_t[:], wpack_d[:, WCOLS: 2 * WCOLS]),
                 18: (bias2[:], bias2_d[:]),
                 20: (bo_t[:], bpack_d[:]),
                 22: (wq_t[:], wpack_d[:, 0: WCOLS])}
        wdmas_late = [(wo_t[:], wpack_d[:, 2 * WCOLS: 3 * WCOLS])]

        def dma_xtg(e, j, tiles, stripes=1, engines=None):
            """Issue the DMA(s) for one quad-packed xT tile [128, 2048]
            covering p-tiles 4j..4j+3 (contiguous in DRAM, so a single
            large DMA per tile keeps the HW DGE queues streaming)."""
            if engines is None:
                engines = (nc.sync,)
            xtt = xtpool.tile([128, 2048], BF16, tag="xt", name=f"xt{e}_{j}")
            w = 2048 // stripes
            for st in range(stripes):
                engines[st % len(engines)].dma_start(
                    xtt[:, st * w:(st + 1) * w],
                    xt_d[e, j, :, st * w:(st + 1) * w])
            tiles[j] = xtt

        def mm_s(Sb, tiles, p):
            """Upper-triangle Gram matmuls for one p-tile. tiles may
            carry a dedicated first-p-tile tile under key -1 (a small
            single-queue DMA that unblocks the very first matmul at the
            earliest possible moment)."""
            if p == 0 and -1 in tiles:
                xtt, base = tiles[-1], 0
            else:
                xtt, base = tiles[p // 4], (p % 4) * 512
            for ci in range(NCT):
                nc.tensor.matmul(Sb[ci][:],
                                 xtt[:, base + 128 * ci: base + 128 * (ci + 1)],
                                 xtt[:, base + 128 * ci: base + 512],
                                 start=(p == 0), stop=(p == NPT - 1))

        def mm_s_tail(Sb, tiles):
            """Last four p-tiles in ci-major order: bank ci stops several
            matmuls before bank ci+1, so hi-strip extraction (ACT)
            overlaps the S tail instead of serializing after it."""
            for ci in range(NCT):
                for p in range(NPT - 4, NPT):
                    base = (p % 4) * 512
                    xtt = tiles[p // 4]
                    nc.tensor.matmul(Sb[ci][:],
                                     xtt[:, base + 128 * ci: base + 128 * (ci + 1)],
                                     xtt[:, base + 128 * ci: base + 512],
                                     start=False, stop=(p == NPT - 1))

        def emit_x_chunk(e, xch, ci, engine):
            """One full-row [128, 4096] load of the [C, P]-layout x
            (epilogue rhs): 1MB with 8KB contiguous lines, one DMA per
            ci so issue-engine time stays low."""
            xt = xpool.tile([128, P], BF16, tag="x", name=f"x{e}_{ci}")
            engine.dma_start(xt[:], x_d[e, ci * 128:(ci + 1) * 128, :])
            xch[ci] = xt

        def emit_s_extract(e, Sb):
            """S extraction (single bf16; rel err ~1.1e-2 vs tolerance
            2e-2 -- the hi/lo split costs 10K PE cycles/example for
            ~2e-3). Strips split across ACT and DVE so extraction
            wall-time halves (both engines can read PSUM; GpSimd can't).
            Split out from the mirror/V stage so the strip chain can be
            issued early and hide under unrelated PE work."""
            Shi = [slvpool.tile([128, C], BF16, tag="slv", name=f"Shi{e}_{ci}")
                   for ci in range(NCT)]
            for ci in range(NCT):
                for s in range(NCT - ci):
                    dsl = slice((ci + s) * 128, (ci + s + 1) * 128)
                    ssl = slice(s * 128, (s + 1) * 128)
                    if (ci + s) % 2 == 0:
                        nc.scalar.activation(Shi[ci][:, dsl], Sb[ci][:, ssl],
                                             IDENT)
                    else:
                        nc.vector.tensor_copy(Shi[ci][:, dsl], Sb[ci][:, ssl])
            return Shi

        def emit_v(e, Shi):
            """Symmetry mirrors + V = S @ wkT + V extraction."""
            def mirror(i, j):
                # S[i-block, j-block] = S[j-block, i-block]^T for j < i
                tp = p2pool.tile([128, 128], BF16, tag="p2",
                                 name=f"mt{e}_{i}{j}")
                nc.tensor.transpose(tp[:], Shi[j][:, 128 * i:128 * (i + 1)],
                                    eye[:])
                nc.vector.tensor_copy(Shi[i][:, 128 * j:128 * (j + 1)], tp[:])

            Vb = [spool.tile([128, 512], F32, tag="sp", name=f"V{e}_{ci}")
                  for ci in range(NCT)]

            def vmm(cj):
                for ci in range(NCT):
                    nc.tensor.matmul(Vb[ci][:],
                                     Shi[cj][:, 128 * ci:128 * (ci + 1)],
                                     wk_t[:, C * cj:C * (cj + 1)],
                                     start=(cj == 0), stop=(cj == NCT - 1))

            mirror(1, 0)
            vmm(0)
            mirror(2, 0)
            mirror(2, 1)
            vmm(1)
            mirror(3, 0)
            mirror(3, 1)
            mirror(3, 2)
            vmm(2)
            vmm(3)

            Vhi = [slvpool.tile([128, C], BF16, tag="slv", name=f"Vhi{e}_{ci}")
                   for ci in range(NCT)]
            for ci in range(NCT):
                for s in range(2):
                    sl = slice(s * 256, (s + 1) * 256)
                    if (ci + s) % 2 == 0:
                        nc.scalar.activation(Vhi[ci][:, sl], Vb[ci][:, sl],
                                             IDENT)
                    else:
                        nc.vector.tensor_copy(Vhi[ci][:, sl], Vb[ci][:, sl])
            return Vhi

        def emit_logits(e, Vhi):
            """Per head-pair logit banks [d, e']: wq^T V + rank-2 bias.
            All four pair banks share one PSUM bank (2KB) as one group.
            The four per-pair rank-2 bias corrections are packed into ONE
            K=8 matmul: k-row 2t+j carries bias row j of pair t, with the
            rhs zeroed outside pair t's 128-col block (block-diagonal)."""
            bt = attpool.tile([128, 512], F32, tag="attp", name=f"Lb{e}")
            banks = [bt[:, t * 128:(t + 1) * 128] for t in range(4)]
            for cj in range(NCT):
                for t in range(4):
                    nc.tensor.matmul(banks[t][:],
                                     wq_t[:, C * cj + 128 * t: C * cj + 128 * (t + 1)],
                                     Vhi[cj][:, 128 * t:128 * (t + 1)],
                                     start=(cj == 0 and t == 0), stop=False)
            nc.tensor.matmul(bt[:],
                             bias2[:, e * 640: e * 640 + 128],
                             bias2[:, e * 640 + 128: (e + 1) * 640],
                             start=False, stop=True)
            return banks

        def emit_att(e, banks):
            """Softmax chain -> normalized att_de tiles. Engines execute
            their queues IN ORDER, so this non-PE part is emitted early
            (its ACT/DVE/Pool ops queue ahead of later conv extractions)
            while the PE-consuming emit_g is emitted late, after the
            conv chunks the chain should hide under. EXPs carry no
            accum_out (the 281ns ACTIVATION_READ_ACCUMULATOR per half
            would double ACT's serial time); Z is a cheap DVE free-dim
            reduce; zero-quadrant fills ride the idle GpSimd."""
            att_des = []
            for t in range(4):
                bank = banks[t]
                pr = pairpool.tile([128, 128], BF16, tag="pair", name=f"pr{e}_{t}")
                z = zpool.tile([128, 1], F32, tag="z", name=f"z{e}_{t}")
                nc.gpsimd.tensor_copy(pr[0:64, 64:128], zblk[0:64, :])
                nc.gpsimd.tensor_copy(pr[64:128, 0:64], zblk[64:128, :])
                nc.scalar.activation(pr[0:64, 0:64], bank[0:64, 0:64], EXP,
                                     scale=0.125, bias=shift[0:64, :])
                nc.scalar.activation(pr[64:128, 64:128], bank[64:128, 64:128],
                                     EXP, scale=0.125, bias=shift[64:128, :])
                nc.vector.tensor_reduce(z[0:64, :], pr[0:64, 0:64],
                                        mybir.AxisListType.X,
                                        mybir.AluOpType.add)
                nc.vector.tensor_reduce(z[64:128, :], pr[64:128, 64:128],
                                        mybir.AxisListType.X,
                                        mybir.AluOpType.add)
                rz = zpool.tile([128, 1], F32, tag="z", name=f"rz{e}_{t}")
                nc.vector.reciprocal(rz[:], z[:])
                att_de = pairpool.tile([128, 128], BF16, tag="pair",
                                       name=f"attde{e}_{t}")
                nc.vector.tensor_scalar_mul(att_de[:], pr[:], rz[:, 0:1])
                att_des.append(att_de)
            return att_des

        def emit_g(e, att_des):
            """G = wo att + I matmuls + extraction, interleaved per pair
            so casts pipeline behind the next pair's matmuls."""
            gs = []
            for t in range(4):
                gp = p2pool.tile([128, 512], F32, tag="p2", name=f"gp{e}_{t}")
                nc.tensor.matmul(gp[:], att_des[t][:],
                                 wo_t[:, C * t:C * (t + 1)],
                                 start=True, stop=False)
                nc.tensor.matmul(gp[:, 128 * t:128 * (t + 1)], eye[:], eye[:],
                                 start=False, stop=True)
                g = gpool.tile([128, C], BF16, tag="g", name=f"g{e}_{t}")
                if t % 2 == 0:
                    nc.vector.tensor_copy(g[:], gp[:])
                else:
                    nc.scalar.activation(g[:], gp[:], IDENT)
                gs.append(g)
            return gs

        def emit_o2rows(e):
            return [o2rpool.tile([128, P], BF16, tag="o2r", name=f"o2r{e}_{co}")
                    for co in range(NCT)]

        def emit_conv_chunk(e, xch, gs, o2rows, p5, ext="alt"):
            """ext='vector' forces PSUM->SBUF extraction onto DVE for
            chunks that ride alongside e1's sv/logits chain, keeping the
            ACT engine free for that chain's strip extractions. e0's
            output drains all issue from Sync so the Scalar engine's
            queue stays clear for e1's softmax chain; e1's split across
            both queues for a short exit tail."""
            drain = (nc.sync, nc.sync) if e == 0 else (nc.sync, nc.scalar)
            sl = slice(p5 * 512, (p5 + 1) * 512)
            for co in range(NCT):
                o2p = p2pool.tile([128, 512], F32, tag="p2",
                                  name=f"o2p{e}_{p5}_{co}")
                for et in range(NCT):
                    nc.tensor.matmul(
                        o2p[:],
                        gs[et][:, co * 128:(co + 1) * 128],
                        xch[et][:, sl],
                        start=(et == 0), stop=(et == NCT - 1))
                # PSUM->SBUF + bias split between DVE and ACT; the very
                # last chunk's parity is flipped so the final co lands
                # on DVE (its chain to the last drain is ~1us shorter
                # than via ACT, which still holds the prior extraction)
                par = (p5 * NCT + co + (1 if e == 1 and p5 == 7 else 0)) % 2
                if ext == "vector" or par == 0:
                    nc.vector.tensor_scalar_add(o2rows[co][:, sl], o2p[:],
                                                bo_t[:, co:co + 1])
                else:
                    nc.scalar.activation(o2rows[co][:, sl], o2p[:], IDENT,
                                         bias=bo_t[:, co:co + 1])
                # e1: last two stages drain per-co right behind the copy
                # so the exit tail is one 64KB transfer per queue deep
                if e == 1 and p5 >= 6:
                    drain[co % 2].dma_start(
                        out_d[e, co * 128:(co + 1) * 128, sl], o2rows[co][:, sl])
            # staged output drains: large per-co DMAs; e0's all ride Sync
            # in two halves (its tail is hidden), e1's stay fine-grained
            # across both queues for a short exit tail
            if e == 0:
                if p5 == 3 or p5 == 7:
                    h = slice(0, 2048) if p5 == 3 else slice(2048, 4096)
                    for co in range(NCT):
                        nc.sync.dma_start(
                            out_d[e, co * 128:(co + 1) * 128, h],
                            o2rows[co][:, h])
            elif p5 == 3:
                for co in range(NCT):
                    drain[co % 2].dma_start(
                        out_d[e, co * 128:(co + 1) * 128, 0:2048],
                        o2rows[co][:, 0:2048])
            elif p5 == 5:
                for co in range(NCT):
                    drain[(co + 1) % 2].dma_start(
                        out_d[e, co * 128:(co + 1) * 128, 2048:3072],
                        o2rows[co][:, 2048:3072])


        # ---- schedule -------------------------------------------------
        # e0 S phase: xT0 rides BOTH hardware DMA queues (sync+scalar),
        # ALL tiles issued upfront so the DGE queues never run dry
        # during the DMA-paced S0 phase; weights ride scalar behind the
        # xt0 stream, x0 late
        Sb0 = [spool.tile([128, 512 - 128 * ci], F32, tag="sp",
                          name=f"S0_{ci}") for ci in range(NCT)]
        xt0 = {}
        xch0 = [None] * NCT
        both = (nc.sync, nc.scalar)
        dma_xtg(0, 0, xt0, stripes=4, engines=both)
        dma_xtg(0, 1, xt0, stripes=2, engines=both)
        # j2..j7 striped on a single queue each: halves complete
        # sequentially, so each tile's first two p-tiles unblock a
        # 256KB transfer earlier (the S0 phase is DMA-paced; 512KB
        # all-or-nothing arrivals stall the PE in bursts)
        for j in range(2, 8):
            dma_xtg(0, j, xt0, stripes=2, engines=(both[j % 2],))
        # Queue-order matters as much as issue time: xT1 j0-j2 go FIRST
        # on Sync's queue (S1 p0 needs j0 at ~34us; behind 4MB of x0 it
        # would land ~37), then x0 split across both queues, then the
        # late weights. Scalar-ENGINE issues sit before any ACT compute
        # (strips come later), so a sem-slot block there is harmless.
        xt1 = {}
        for p in range(NPT - 4):
            mm_s(Sb0, xt0, p)
            if p in wdmas:
                dst, src = wdmas.pop(p)
                nc.scalar.dma_start(dst, src)
            if 21 <= p < 24:
                dma_xtg(1, p - 21, xt1)
            elif p == 24:
                # wo ahead of x0 on Sync's queue: the G0 matmuls (which
                # gate gs0 and thus conv0's start) need wo by ~43.5us,
                # while x0's first use (conv0 chunk 0) is ~46.5us
                nc.sync.dma_start(*wdmas_late[0])
            elif p == 25:
                emit_x_chunk(0, xch0, 0, nc.sync)
            elif p == 26 or p == 27:
                emit_x_chunk(0, xch0, p - 25, nc.scalar)
        # S0 tail in ci-major order with strips right behind: bank ci's
        # extraction starts as soon as it stops, ahead of everything
        # else in ACT's in-order queue
        mm_s_tail(Sb0, xt0)
        Shi0 = emit_s_extract(0, Sb0)
        # last x0 row on Sync (no-blocking rule: never put a blockable
        # DMA issue on Scalar once ACT compute is queued)
        emit_x_chunk(0, xch0, 3, nc.sync)

        Vhi0 = emit_v(0, Shi0)
        Sb1 = [spool.tile([128, 512 - 128 * ci], F32, tag="sp",
                          name=f"S1_{ci}") for ci in range(NCT)]
        for p in range(0, 6):
            mm_s(Sb1, xt1, p)
        banks0 = emit_logits(0, Vhi0)
        att0 = emit_att(0, banks0)
        for p in range(6, 12):
            mm_s(Sb1, xt1, p)
        gs0 = emit_g(0, att0)

        # conv0 chunks interleaved with remaining e1 S tiles + x1 loads;
        # e1's extraction/logits/softmax chains each ride behind a conv0
        # chunk so their serial ACT/DVE latency hides under PE work
        o2r0 = emit_o2rows(0)
        xch1 = [None] * NCT
        for j in range(3, 6):
            dma_xtg(1, j, xt1)
        p1 = 12
        x1_next = 0
        sched = [6, 6, 4, 0, 0, 0, 0, 0]
        for i in range(NP5):
            if i == 3:
                # S1 tail in ci-major order (bank ci stops early) so the
                # strip chain starts before the conv chunk and hides
                # under conv3/conv4's PE work; strips are queued on
                # ACT/DVE ahead of conv3's extraction ops
                mm_s_tail(Sb1, xt1)
                Shi1 = emit_s_extract(1, Sb1)
            # chunks riding alongside e1's sv chain extract on DVE only,
            # so ACT's queue stays clear for the chain's strips
            emit_conv_chunk(0, xch0, gs0, o2r0, i,
                            ext="vector" if 3 <= i <= 4 else "alt")
            if i < 2:
                dma_xtg(1, 6 + i, xt1)
            for _ in range(sched[i]):
                if p1 < NPT - 4:
                    mm_s(Sb1, xt1, p1)
                    p1 += 1
            if x1_next < 4:
                emit_x_chunk(1, xch1, x1_next, nc.sync)
                x1_next += 1
            if i == 4:
                Vhi1 = emit_v(1, Shi1)
                banks1 = emit_logits(1, Vhi1)
            elif i == 5:
                # non-PE softmax chain only: its ACT/DVE ops queue ahead
                # of conv5-7's extractions; the PE-consuming G matmuls
                # are emitted after conv7 so PE never blocks on att_de
                att1 = emit_att(1, banks1)

        gs1 = emit_g(1, att1)
        o2r1 = emit_o2rows(1)
        for i in range(NP5):
            emit_conv_chunk(1, xch1, gs1, o2r1, i)

    nc.compile()
    return nc


_NC_CACHE = None


def _get_nc():
    global _NC_CACHE
    if _NC_CACHE is None:
        _NC_CACHE = build_nc()
    return _NC_CACHE


def make_in_maps(inputs):
    x = np.ascontiguousarray(np.asarray(inputs["x"], dtype=np.float32))
    wq = np.asarray(inputs["wq"], dtype=np.float32)
    wk = np.asarray(inputs["wk"], dtype=np.float32)
    wo = np.asarray(inputs["wo"], dtype=np.float32)
    bq = np.asarray(inputs["bq"], dtype=np.float32)
    bk = np.asarray(inputs["bk"], dtype=np.float32)
    bo = np.asarray(inputs["bo"], dtype=np.float32)

    x32 = x.reshape(B, C, P)
    xr = x32.astype(BF)                                   # [B, C, P] bf16
    xtr = np.ascontiguousarray(xr.transpose(0, 2, 1))     # [B, P, C] bf16
    # quad-packed xT: [B, NPG//2, 128, 2048], tile j = p-tiles 4j..4j+3
    xt4 = np.ascontiguousarray(
        xtr.reshape(B, NPG // 2, 4, 128, C).transpose(0, 1, 3, 2, 4)
           .reshape(B, NPG // 2, 128, 2048))

    # rank-2 bias-correction rows (exact f32 host math)
    s = x32.sum(axis=2)                                   # [B, C]
    qs = s @ wq.T                                         # [B, C]
    ks = s @ wk.T                                         # [B, C]

    wpack = np.zeros((128, 3 * WCOLS + 192), dtype=BF)
    for i, w in enumerate((wq, wk, wo)):
        wt = w.T.astype(BF)  # [ci, co]
        for ci in range(NCT):
            wpack[:, i * WCOLS + ci * C: i * WCOLS + (ci + 1) * C] = \
                wt[ci * 128:(ci + 1) * 128, :]
    ko = 3 * WCOLS
    wpack[:, ko + 64: ko + 192] = np.eye(128, dtype=np.float32).astype(BF)

    bpack = bo.reshape(NCT, 128).T.astype(np.float32)
    bpack = np.ascontiguousarray(bpack)

    in_maps = []
    for cix in range(NCORES):
        # K=8 packed rank-2 bias rows: k-row 2t+j holds bias row j of
        # head-pair t; lhsT cols are the pair's 128 d-rows, rhs cols are
        # zero outside the pair's 128-col block (block-diagonal).
        bias2 = np.zeros((8, BL * 640), dtype=BF)
        for e in range(BL):
            ge = cix * BL + e
            a = np.stack([bq, qs[ge]])                 # [2, C] lhsT rows
            b = np.stack([ks[ge] + P * bk, bk])        # [2, C] rhs rows
            for t in range(4):
                for j in range(2):
                    k = 2 * t + j
                    bias2[k, e * 640: e * 640 + 128] = \
                        a[j, t * 128:(t + 1) * 128].astype(BF)
                    bias2[k, e * 640 + 128 + t * 128: e * 640 + 128 + (t + 1) * 128] = \
                        b[j, t * 128:(t + 1) * 128].astype(BF)
        in_maps.append({
            "x": np.ascontiguousarray(xr[cix * BL: (cix + 1) * BL]),
            "xt": np.ascontiguousarray(xt4[cix * BL: (cix + 1) * BL]),
            "wpack": wpack, "bias2": bias2, "bpack": bpack,
        })
    return in_maps


def run_sharded(inputs, trace=False, **kw):
    nc = _get_nc()
    in_maps = make_in_maps(inputs)
    res = bass_utils.run_bass_kernel_spmd(
        nc, in_maps, core_ids=list(range(NCORES)), trace=trace, **kw
    )
    outs = [np.asarray(res.results[i]["out"]).astype(np.float32)
            for i in range(NCORES)]
    full = np.concatenate(outs, axis=0).reshape(B, C, HH, WW)
    return full.astype(np.float32), res


def kernel(**inputs):
    out, _ = run_sharded(inputs, trace=False)
    return out

